# revision 73
# baseline (speedup 1.0000x reference)
"""TRN2 Bass kernel for nn_Attention_35854386987650.

Single-block attention: QKV projection of x[1,1024,1024], KV-cache update at
pos=0, softmax over 1025 visible slots (1024 fresh + cache slot 1024), output
projection. Head-parallel across 8 NeuronCores (1 head per core); the
row-parallel output projection partials are summed on the host.

Per-core layout strategy (head h):
  - host pre-transposes x -> xT [e, i] in fp16 (same DMA bytes and PE rate
    as bf16, 4x the mantissa); Wq|Wk host-packed interleaved into one fp16
    tensor, Wv packed fp16, Wo row-slice bf16. All input DMAs issue on the
    SP and Pool (SWDGE) queues so the ACT queue only carries the
    activation-table load and stays free for the exp stream.
  - QT/KT computed in [d, i] layout (fp16 matmuls, f32 PSUM accumulate),
    h0 columns of both finish ~2.5us before h1 so the first scores and
    the serial ACT exp stream start early; evacuated to f32 on DVE/ACT.
  - V computed directly in [token, d] layout (lhsT = xT chunk, rhs = Wv
    chunk) as an interleavable PE filler stream - no PE transposes or
    identity tile; bv folds in as a 1-partition ones-row x bv-row matmul
    in the same PSUM accumulation group (skipped when biases are zero).
  - scores ST_j[j, i] = KT[:,j]^T @ QT; exp on ACT, bf16 out (P~ up to
    e^55 needs bf16 range; no max subtraction, logits are safe in f32).
    All h0 exps run before any h1 exp so the h0 output projection
    overlaps the h1 exp stream.
  - softmax denominators via tiny accumulating PE matmuls (P~ slice x
    ones column). PSUM zero-regions are 2KB-bank-granular, so each
    half's four columns form ONE accumulation group in its own bank;
    pden0/po0/po1/pden1 rotate through two "po"-tag slots behind
    psq0/psq1, freed just in time by the qt evacuations and ot0.
  - O^T[d, i] = sum_j V_j @ P~_j (bf16); Y_t[i, n] = (O^T[:, t])^T @ Wo,
    1/den applied at evacuation (DVE mid-stream, ACT after its last exp;
    GPSIMD cannot read PSUM on real hw). y partials leave in fp16 on
    SP/Pool/ACT queues, the last two tiles split across two queues; the
    host accumulates the 8 partials in f64.
  - cache slot T: the caches produced by setup_inputs() are all-zero, so
    den += 1 (fast variant). A general variant handles nonzero caches via
    a 9th key tile (k9/v9 with a -1e30 exp-bias) picked automatically.
"""
import sys

if "/opt/trn_rl_repo" not in sys.path:
    sys.path.insert(0, "/opt/trn_rl_repo")

import numpy as np

import concourse.bass as bass  # noqa: F401  (bass must import before bacc)
from concourse import bacc, mybir
import concourse.tile as tile
from concourse import bass_utils

T = 1024       # sequence length
D = 1024       # embed dim
HD = 128       # head dim
NCORES = 8
EC = D // 128  # contraction chunks over embed dim
JT = T // 128  # key tiles
IT = T // 128  # query tiles
MASK = -1.0e30

F32 = mybir.dt.float32
F32R = mybir.dt.float32r
BF16 = mybir.dt.bfloat16
F16 = mybir.dt.float16
EXP = mybir.ActivationFunctionType.Exp
COPY = mybir.ActivationFunctionType.Copy
IDENT = mybir.ActivationFunctionType.Identity

# misc f32 tensor column layout: k9 | bq | bk | mask9
MF_K9 = 0
MF_BQ = 128
MF_BK = 129
MF_MASK = 130
MF_COLS = 131

# misc bf16 tensor column layout: v9 | ones_col | ones_row | bv_row | ones_row_bf
MB_V9 = 0
MB_ONESC = 128
MB_ONESR = 129
MB_BVR = 257
MB_ONESB = 385
MB_COLS = 517

_CACHED = {}

# tail engine/DMA assignment: (eng_a, eng_b, dma_a[, dma_b]) per tile for
# yt(0,3), yt(1,0..3); engines 0=ACT 1=DVE; dma 0=ACT 1=SP 2=Pool
TAIL_CFG = [(0, 0, 2), (0, 1, 2), (1, 0, 1), (0, 1, 1, 2), (1, 0, 1, 0)]


def _build(variant):
    with_cache_tile, with_bias = variant
    nc = bacc.Bacc(None, target_bir_lowering=False)

    xt_d = nc.dram_tensor("xt", [D, T], F16, kind="ExternalInput")      # x^T
    # wq/wk packed interleaved: [wqA | wkA | wqB | wkB], 512 cols each
    wqk_d = nc.dram_tensor("wqk", [128, 2 * D], F16, kind="ExternalInput")
    wv_d = nc.dram_tensor("wv", [128, D], F16, kind="ExternalInput")
    wo_d = nc.dram_tensor("wo", [HD, D], BF16, kind="ExternalInput")     # row slice
    mf_d = nc.dram_tensor("miscf", [128, MF_COLS], F32, kind="ExternalInput")
    mb_d = nc.dram_tensor("miscb", [128, MB_COLS], BF16, kind="ExternalInput")
    # partial output in bf16: each core's partial is rounded once; the host
    # accumulates the 8 partials in f64 (within tolerance, and halves the
    # 4MB output-DMA tail)
    y_d = nc.dram_tensor("y", [T, D], F16, kind="ExternalOutput")

    njt = JT + 1 if with_cache_tile else JT

    with tile.TileContext(nc) as tc:
        with (
            tc.tile_pool(name="sb", bufs=1) as sb,
            tc.tile_pool(name="yout", bufs=8) as yp,
            tc.tile_pool(name="mm", bufs=1, space="PSUM") as pmm,
            tc.tile_pool(name="pox", bufs=1, space="PSUM") as ppo,
        ):
            # ---- input loads: SP + Pool queues only ----
            xts = [None] * EC

            def load_xt(c, eng):
                xtile = sb.tile([128, T], F16, tag=f"xt{c}")
                eng.dma_start(out=xtile, in_=xt_d.ap()[c * 128:(c + 1) * 128, :])
                xts[c] = xtile

            wqka = sb.tile([128, 1024], F16, tag="wqka")
            wqkb = sb.tile([128, 1024], F16, tag="wqkb")
            wqa, wka = wqka[:, 0:512], wqka[:, 512:1024]
            wqb, wkb = wqkb[:, 0:512], wqkb[:, 512:1024]
            wv = sb.tile([128, D], F16, tag="wv")
            wo = sb.tile([HD, D], BF16, tag="wo")
            mf = sb.tile([128, MF_COLS], F32, tag="mf")
            mb = sb.tile([128, MB_COLS], BF16, tag="mb")

            # SP queue: wqA, wkA, xt1, xt3, xt5, xt7, wo (wqA alone first
            # so the first projection matmul starts at the DMA-latency floor)
            nc.sync.dma_start(out=wqka[:, 0:512], in_=wqk_d.ap()[:, 0:512])
            nc.sync.dma_start(out=wqka[:, 512:1024],
                              in_=wqk_d.ap()[:, 512:1024])
            load_xt(1, nc.sync)
            load_xt(3, nc.sync)
            load_xt(5, nc.sync)
            load_xt(7, nc.sync)
            nc.sync.dma_start(out=wo, in_=wo_d.ap())
            # Pool queue: xt0, xt2, wqB, wkB, xt4, xt6, wv, miscb, miscf
            load_xt(0, nc.gpsimd)
            load_xt(2, nc.gpsimd)
            nc.gpsimd.dma_start(out=wqkb, in_=wqk_d.ap()[:, 1024:2048])
            load_xt(4, nc.gpsimd)
            load_xt(6, nc.gpsimd)
            nc.gpsimd.dma_start(out=wv, in_=wv_d.ap())
            nc.gpsimd.dma_start(out=mb, in_=mb_d.ap())
            nc.gpsimd.dma_start(out=mf, in_=mf_d.ap())

            def wqh(c):
                t = wqa if c < 4 else wqb
                return t[:, (c % 4) * 128:(c % 4 + 1) * 128]

            def wkh(c):
                t = wka if c < 4 else wkb
                return t[:, (c % 4) * 128:(c % 4 + 1) * 128]

            k9 = mf[:, MF_K9:MF_K9 + 128].bitcast(F32R)
            bq = mf[:, MF_BQ:MF_BQ + 1]
            bk = mf[:, MF_BK:MF_BK + 1]
            mask9 = mf[:, MF_MASK:MF_MASK + 1]
            v9 = mb[:, MB_V9:MB_V9 + 128]
            ones_c = mb[:, MB_ONESC:MB_ONESC + 1]
            ones_r = mb[0:1, MB_ONESR:MB_ONESR + 128].bitcast(F16)
            bv_r = mb[0:1, MB_BVR:MB_BVR + 128].bitcast(F16)
            ones_rb = mb[0:1, MB_ONESB:MB_ONESB + 128]

            # ---- Q/K projections: [d, i] = sum_c W_c^T @ xT_c ----
            # h0 (columns 0:512) of both Q and K runs first so the first
            # scores and the ACT exp stream start ~2.5us before the h1
            # projections are done; h1 matmuls fill PE while waiting for
            # the last x chunks to land
            psq0 = ppo.tile([128, 512], F32, tag="po", bufs=2)
            psq1 = ppo.tile([128, 512], F32, tag="po", bufs=2)
            psk0 = pmm.tile([128, 512], F32, tag="qk", bufs=3)
            psk1 = pmm.tile([128, 512], F32, tag="qk", bufs=3)

            def proj(ps, w, c, half, st0, sp):
                nc.tensor.matmul(ps, w(c), xts[c][:, half * 512:(half + 1) * 512],
                                 start=st0, stop=sp)

            for c in range(EC - 1):
                proj(psq0, wqh, c, 0, c == 0, False)
                proj(psk0, wkh, c, 0, c == 0, False)
            proj(psq1, wqh, 0, 1, True, False)
            proj(psk1, wkh, 0, 1, True, False)
            proj(psq1, wqh, 1, 1, False, False)
            proj(psk1, wkh, 1, 1, False, False)
            proj(psq0, wqh, EC - 1, 0, False, True)
            proj(psk0, wkh, EC - 1, 0, False, True)

            qt = sb.tile([HD, T], F32R, tag="qt")
            kt = sb.tile([HD, T], F32R, tag="kt")

            def evq(dst, src, act=False):
                if act:
                    # ACT is idle until the first exp; Identity takes a bias
                    nc.scalar.activation(dst, src, IDENT, bias=bq) \
                        if with_bias else \
                        nc.scalar.activation(dst, src, COPY)
                elif with_bias:
                    nc.vector.tensor_scalar_add(dst, src, bq)
                else:
                    nc.vector.tensor_copy(dst, src)

            def evk(dst, src):
                if with_bias:
                    nc.vector.tensor_scalar_add(dst, src, bk)
                else:
                    nc.vector.tensor_copy(dst, src)

            evq(qt[:, 0:512], psq0, act=True)
            evk(kt[:, 0:128], psk0[:, 0:128])
            evk(kt[:, 128:512], psk0[:, 128:512])
            # K h1 finishes before Q h1 and its evacuation goes first, so
            # the later h0 scores (which need kt columns 512+) unblock early
            for c in range(2, EC):
                proj(psk1, wkh, c, 1, False, c == EC - 1)
            evk(kt[:, 512:1024], psk1)

            # ---- attention helpers ----
            jorder = ([JT] if with_cache_tile else []) + list(range(JT))
            pts = {0: [None] * (JT + 1), 1: [None] * (JT + 1)}
            vjs = {JT: v9}

            def _vgen():
                # V matmuls as an interleavable filler stream: each token
                # tile is an accumulation group of 8 (+bias) 128-free mms;
                # other matmuls may interleave between them freely
                for t in range(JT):
                    vtag, vb = ("qk", 3) if t < 6 else ("st", 3)
                    psv = pmm.tile([128, HD], F32, tag=vtag, bufs=vb,
                                   padded_shape=[128, 512], name=f"psv{t}")
                    for c in range(EC):
                        nc.tensor.matmul(psv, xts[c][:, t * 128:(t + 1) * 128],
                                         wv[:, c * 128:(c + 1) * 128],
                                         start=(c == 0),
                                         stop=(not with_bias and c == EC - 1))
                        if c < EC - 1:
                            yield
                    if with_bias:
                        nc.tensor.matmul(psv, ones_r, bv_r,
                                         start=False, stop=True)
                    vj = sb.tile([128, HD], BF16, tag=f"vj{t}", name=f"vj{t}")
                    # GPSIMD cannot read PSUM on real hw - DVE evacuates
                    nc.vector.tensor_copy(vj, psv)
                    vjs[t] = vj
                    yield
                while True:
                    yield

            vgen = _vgen()

            def vfill(n):
                for _ in range(n):
                    next(vgen)

            def st_exp(H, j):
                hs = slice(H * 512, (H + 1) * 512)
                lhsT = k9 if j == JT else kt[:, j * 128:(j + 1) * 128]
                ps = pmm.tile([128, 512], F32, tag="st", bufs=3)
                nc.tensor.matmul(ps, lhsT, qt[:, hs], start=True, stop=True)
                pt = sb.tile([128, 512], BF16, tag=f"pt{j}h{H}")
                if j == JT:
                    nc.scalar.activation(pt, ps, EXP, bias=mask9)
                else:
                    nc.scalar.activation(pt, ps, EXP)
                pts[H][j] = pt

            # psum zero-regions are 2KB-bank-granular: each half's four
            # denominator columns form ONE accumulation group in its own
            # bank (h1 reuses po0's bank, free after ot0's evacuation)
            pden0 = ppo.tile([128, IT // 2], F32, tag="po", bufs=2,
                             name="pden0")
            pdens = [pden0, None]

            def pv_den(H, po, idx):
                j = jorder[idx]
                nc.tensor.matmul(po, vjs[j], pts[H][j],
                                 start=(idx == 0), stop=(idx == njt - 1))
                if H == 1 and pdens[1] is None:
                    pdens[1] = ppo.tile([128, IT // 2], F32, tag="po",
                                        bufs=2, name="pden1")
                pden = pdens[H]
                for q in range(4):
                    nc.tensor.matmul(pden[:, q:q + 1],
                                     pts[H][j][:, q * 128:(q + 1) * 128],
                                     ones_c,
                                     start=(idx == 0 and q == 0),
                                     stop=(idx == njt - 1 and q == 3))
                if idx == 0 and not with_cache_tile:
                    # cache slot contributes exactly exp(0)=1: fold the +1
                    # into the accumulation (out[p,q] += 1*1) so den_recip
                    # is a bare reciprocal off the critical tail chain
                    nc.tensor.matmul(pden[:, 0:4], ones_rb[:, 0:128],
                                     mb[0:1, MB_ONESB:MB_ONESB + 4],
                                     start=False, stop=False)

            def den_recip(H):
                denrt = sb.tile([128, IT // 2], F32, tag=f"denrt{H}")
                nc.vector.reciprocal(denrt, pdens[H][:, 0:4])
                return denrt

            def ot_evac(H, po):
                ot = sb.tile([HD, 512], BF16, tag=f"ot{H}")
                # staggered: the first Y matmul only needs the first 128
                # columns; for h1 ACT (free right after its last exp)
                # takes the big slice so the DVE tail shortens
                nc.vector.tensor_copy(ot[:, 0:128], po[:, 0:128])
                nc.vector.tensor_copy(ot[:, 128:512], po[:, 128:512])
                return ot

            # y evac engines per (tile, half): 0=ACT 1=DVE
            # (GPSIMD cannot access PSUM on real hw)
            def yev(eng, dst, src, scale):
                if eng == 0:
                    nc.scalar.activation(dst, src, COPY, scale=scale)
                else:
                    nc.vector.tensor_scalar_mul(dst, src, scale)

            DMAE = {0: nc.scalar, 1: nc.sync, 2: nc.gpsimd}

            def ytile_mm(H, t4i, ot):
                # h0 Y tiles rotate through the V/psk slots, h1 alternates
                # between the qk and st slots (their previous tenants are
                # free by then) so evac turnarounds overlap
                tagsel = 0 if H == 0 else t4i % 2 + 1
                tag, nb = [("qk", 3), ("qk", 3), ("st", 3)][tagsel]
                pa = pmm.tile([128, 512], F32, tag=tag, bufs=nb, name="pa")
                pb = pmm.tile([128, 512], F32, tag=tag, bufs=nb, name="pb")
                lhsT = ot[:, t4i * 128:(t4i + 1) * 128]
                nc.tensor.matmul(pa, lhsT, wo[:, 0:512], start=True, stop=True)
                nc.tensor.matmul(pb, lhsT, wo[:, 512:1024], start=True, stop=True)
                return (H, t4i, pa, pb)

            def ytile_fin(hmm, denrt, eng_a, eng_b, dma_a, dma_b=None):
                H, t4i, pa, pb = hmm
                t = H * 4 + t4i
                yt = yp.tile([128, D], F16, tag="y")
                scale = denrt[:, t4i:t4i + 1]
                yev(eng_a, yt[:, 0:512], pa, scale)
                yev(eng_b, yt[:, 512:1024], pb, scale)
                rows = y_d.ap()[t * 128:(t + 1) * 128, :]
                if dma_b is not None:
                    # tail tiles: halves on two queues so the final
                    # transfer's fixed overhead isn't fully exposed
                    DMAE[dma_a].dma_start(out=rows[:, 0:512], in_=yt[:, 0:512])
                    DMAE[dma_b].dma_start(out=rows[:, 512:1024],
                                          in_=yt[:, 512:1024])
                else:
                    DMAE[dma_a].dma_start(out=rows, in_=yt)

            def ytile(H, t4i, ot, denrt, eng_a, eng_b, dma, split_dma=False):
                hmm = ytile_mm(H, t4i, ot)
                if split_dma:
                    ytile_fin(hmm, denrt, eng_a, eng_b, 1, 0)
                else:
                    ytile_fin(hmm, denrt, eng_a, eng_b, dma)

            # ---- emission order (PE stream) ----
            # h0 scores/exps start while the h1 projections and V tiles
            # still fill PE; all h0 exps run on ACT before the h1 exps, so
            # the h0 output projection overlaps the h1 exp stream and only
            # the h1 tail chain is exposed at the end.
            po0 = ppo.tile([HD, 512], F32, tag="po", bufs=2,
                           padded_shape=[128, 512])
            po1 = ppo.tile([HD, 512], F32, tag="po", bufs=2,
                           padded_shape=[128, 512])

            kst = [0, 0]
            kpv = [0, 0]

            def st0():
                st_exp(0, jorder[kst[0]])
                kst[0] += 1

            def st1():
                st_exp(1, jorder[kst[1]])
                kst[1] += 1

            def pv0():
                pv_den(0, po0, kpv[0])
                kpv[0] += 1

            def pv1():
                pv_den(1, po1, kpv[1])
                kpv[1] += 1

            if with_cache_tile:
                st0()
            st0()                                   # st(0,0)
            st0()                                   # st(0,1)
            proj(psq1, wqh, 2, 1, False, False)
            st0()                                   # st(0,2)
            vfill(8)                                # V0
            if with_cache_tile:
                pv0()
            pv0()                                   # pv(0,0)
            proj(psq1, wqh, 3, 1, False, False)
            st0()                                   # st(0,3)
            vfill(8)                                # V1
            pv0()                                   # pv(0,1)
            proj(psq1, wqh, 4, 1, False, False)
            st0()                                   # st(0,4)
            vfill(8)                                # V2
            pv0()                                   # pv(0,2)
            proj(psq1, wqh, 5, 1, False, False)
            st0()                                   # st(0,5)
            vfill(8)                                # V3
            pv0()                                   # pv(0,3)
            proj(psq1, wqh, 6, 1, False, False)
            st0()                                   # st(0,6)
            vfill(8)                                # V4
            pv0()                                   # pv(0,4)
            proj(psq1, wqh, 7, 1, False, True)
            evq(qt[:, 512:1024], psq1)
            st0()                                   # st(0,7)
            vfill(8)                                # V5
            pv0()                                   # pv(0,5)
            vfill(8)                                # V6
            pv0()                                   # pv(0,6)
            vfill(8)                                # V7
            pv0()                                   # pv(0,7)
            ot0 = ot_evac(0, po0)
            denrt0 = den_recip(0)

            # ---- half 1: h1 scores/exps/PV with h0 Y tiles as filler ----
            if with_cache_tile:
                st1()
            st1()                                   # st(1,0)
            st1()                                   # st(1,1)
            st1()                                   # st(1,2)
            ytile(0, 0, ot0, denrt0, 1, 1, 1)
            if with_cache_tile:
                pv1()
            pv1()                                   # pv(1,0)
            st1()                                   # st(1,3)
            ytile(0, 1, ot0, denrt0, 1, 1, 2)
            pv1()                                   # pv(1,1)
            st1()                                   # st(1,4)
            ytile(0, 2, ot0, denrt0, 1, 1, 1)
            pv1()                                   # pv(1,2)
            st1()                                   # st(1,5)
            pv1()                                   # pv(1,3)
            st1()                                   # st(1,6)
            y03 = ytile_mm(0, 3, ot0)
            pv1()                                   # pv(1,4)
            st1()                                   # st(1,7)
            pv1()                                   # pv(1,5)
            pv1()                                   # pv(1,6)
            pv1()                                   # pv(1,7)
            ot1 = ot_evac(1, po1)
            denrt1 = den_recip(1)
            # yt(0,3) finish deferred here so denrt1/ot1 go first in the
            # DVE queue; ACT takes it right after its last exp
            c03, c10, c11, c12, c13 = TAIL_CFG
            ytile_fin(y03, denrt0, *c03)
            y10 = ytile_mm(1, 0, ot1)
            ytile_fin(y10, denrt1, *c10)
            y11 = ytile_mm(1, 1, ot1)
            ytile_fin(y11, denrt1, *c11)
            y12 = ytile_mm(1, 2, ot1)
            ytile_fin(y12, denrt1, *c12)
            y13 = ytile_mm(1, 3, ot1)
            ytile_fin(y13, denrt1, *c13)

    nc.finalize()
    return nc


def get_nc(variant=(False, False)):
    if variant not in _CACHED:
        _CACHED[variant] = _build(variant)
    return _CACHED[variant]


def _pack_w(W, h):
    """[1024, 128] head slice -> [128, 8*128]: out[p, c*128+d] = W[c*128+p, hd+d]."""
    sl = W[:, h * HD:(h + 1) * HD]                      # [1024, 128]
    return np.ascontiguousarray(
        sl.reshape(EC, 128, HD).transpose(1, 0, 2).reshape(128, EC * HD))


def _bf(a):
    import ml_dtypes
    return np.asarray(a, ml_dtypes.bfloat16)


def _f16(a):
    return np.asarray(a, np.float16)


def make_in_maps(x, Wq, bq, Wk, bk, Wv, bv, Wo, bo, key_cache, value_cache):
    import ml_dtypes
    xt = np.ascontiguousarray(np.asarray(x, np.float32).reshape(T, D).T)
    Wq = np.asarray(Wq, np.float32)
    Wk = np.asarray(Wk, np.float32)
    Wv = np.asarray(Wv, np.float32)
    Wo = np.asarray(Wo, np.float32)
    bq = np.asarray(bq, np.float32)
    bk = np.asarray(bk, np.float32)
    bv = np.asarray(bv, np.float32)
    kc = np.asarray(key_cache, np.float32)
    vc = np.asarray(value_cache, np.float32)
    xt_b = _f16(xt)
    in_maps = []
    for h in range(NCORES):
        sl = slice(h * HD, (h + 1) * HD)
        mf = np.zeros((128, MF_COLS), np.float32)
        mf[:, MF_K9] = kc[0, T, h, :]
        mf[:, MF_BQ] = bq[sl]
        mf[:, MF_BK] = bk[sl]
        mf[1:, MF_MASK] = MASK
        mbf = np.zeros((128, MB_COLS), np.float32)
        mbf[0, MB_V9:MB_V9 + 128] = vc[0, T, h, :]
        mbf[:, MB_ONESC] = 1.0
        mbf16 = _bf(mbf)
        # ones_r/bv_r carry fp16 bit patterns inside the bf16 tensor
        mbf16[0, MB_ONESR:MB_ONESR + 128] = _f16(
            np.ones(128, np.float32)).view(np.uint16).view(ml_dtypes.bfloat16)
        mbf16[0, MB_BVR:MB_BVR + 128] = _f16(
            bv[sl]).view(np.uint16).view(ml_dtypes.bfloat16)
        mbf16[0, MB_ONESB:MB_ONESB + 128] = 1.0
        wq_p = _pack_w(Wq, h)
        wk_p = _pack_w(Wk, h)
        wqk = np.concatenate(
            [wq_p[:, 0:512], wk_p[:, 0:512],
             wq_p[:, 512:1024], wk_p[:, 512:1024]], axis=1)
        in_maps.append({
            "xt": xt_b,
            "wqk": _f16(wqk),
            "wv": _f16(_pack_w(Wv, h)),
            "wo": _bf(np.ascontiguousarray(Wo[sl, :])),
            "miscf": mf,
            "miscb": mbf16,
        })
    return in_maps


_RUNNERS = {}


def _make_runner(nc):
    """Cached analog of bass2jax.run_bass_via_pjrt: builds the sharded jit
    callable once so repeat kernel() calls skip retracing/recompiling."""
    import jax
    from jax.experimental.shard_map import shard_map
    from jax.sharding import Mesh, PartitionSpec
    from concourse import mybir as mb
    from concourse.bass2jax import (_bass_exec_p, install_neuronx_cc_hook,
                                    partition_id_tensor)

    install_neuronx_cc_hook()
    partition_name = (nc.partition_id_tensor.name
                      if nc.partition_id_tensor else None)
    in_names, out_names, out_avals, zero_outs = [], [], [], []
    for alloc in nc.m.functions[0].allocations:
        if not isinstance(alloc, mb.MemoryLocationSet):
            continue
        name = alloc.memorylocations[0].name
        if alloc.kind == "ExternalInput":
            if name != partition_name:
                in_names.append(name)
        elif alloc.kind == "ExternalOutput":
            shape = tuple(alloc.tensor_shape)
            dtype = mb.dt.np(alloc.dtype)
            out_names.append(name)
            out_avals.append(jax.core.ShapedArray(shape, dtype))
            zero_outs.append(np.zeros(shape, dtype))
    n_params = len(in_names)
    all_names = in_names + out_names
    if partition_name is not None:
        all_names = all_names + [partition_name]
    donate = tuple(range(n_params, n_params + len(out_names)))

    def _body(*args):
        operands = list(args)
        if partition_name is not None:
            operands.append(partition_id_tensor())
        return tuple(_bass_exec_p.bind(
            *operands,
            out_avals=tuple(out_avals),
            in_names=tuple(all_names),
            out_names=tuple(out_names),
            lowering_input_output_aliases=(),
            sim_require_finite=True,
            sim_require_nnan=True,
            nc=nc,
        ))

    devices = jax.devices()[:NCORES]
    mesh = Mesh(np.asarray(devices), ("core",))
    nio = n_params + len(out_names)
    sharded = jax.jit(
        shard_map(_body, mesh=mesh,
                  in_specs=(PartitionSpec("core"),) * nio,
                  out_specs=(PartitionSpec("core"),) * len(out_names),
                  check_rep=False),
        donate_argnums=donate, keep_unused=True)

    def run(in_maps):
        concat_in = [
            np.concatenate([np.asarray(m[nm]) for m in in_maps], axis=0)
            for nm in in_names]
        concat_zeros = [
            np.zeros((NCORES * z.shape[0], *z.shape[1:]), z.dtype)
            for z in zero_outs]
        outs = sharded(*concat_in, *concat_zeros)
        return [
            {nm: np.asarray(outs[i]).reshape(NCORES, *out_avals[i].shape)[c]
             for i, nm in enumerate(out_names)}
            for c in range(NCORES)]

    return run


def _run(nc, in_maps, variant):
    runner = _RUNNERS.get(variant, "unset")
    if runner == "unset":
        try:
            runner = _make_runner(nc)
        except Exception:
            runner = None
        _RUNNERS[variant] = runner
    if runner is not None:
        try:
            return runner(in_maps)
        except Exception:
            _RUNNERS[variant] = None
    res = bass_utils.run_bass_kernel_spmd(nc, in_maps,
                                          core_ids=list(range(NCORES)))
    return res.results


def kernel(x, Wq, bq, Wk, bk, Wv, bv, Wo, bo, key_cache, value_cache, pos):
    assert int(np.asarray(pos)) == 0, "kernel hardcodes pos=0"
    in_maps = make_in_maps(x, Wq, bq, Wk, bk, Wv, bv, Wo, bo,
                           key_cache, value_cache)
    kc = np.asarray(key_cache, np.float32)[0, T, :, :]
    vc = np.asarray(value_cache, np.float32)[0, T, :, :]
    with_cache_tile = bool(np.any(kc) or np.any(vc))
    with_bias = bool(np.any(np.asarray(bq)) or np.any(np.asarray(bk))
                     or np.any(np.asarray(bv)))
    variant = (with_cache_tile, with_bias)
    nc = get_nc(variant)
    results = _run(nc, in_maps, variant)
    y = results[0]["y"].astype(np.float64)
    for r in results[1:]:
        y = y + r["y"].astype(np.float64)
    y = y + np.asarray(bo, np.float32).astype(np.float64)[None, :]
    return y.reshape(1, T, D).astype(np.float32)
